# revision 55
# baseline (speedup 1.0000x reference)
"""Trainium2 Bass kernel for nn_BaseTimeAttention (dense transformer block:
QKV projection + RoPE + softmax attention + output projection).

Problem (hardcoded):
  x:  [B=2, S=2048, H=2048] fp32,  Wq/Wk/Wv/Wo: [2048, 2048] fp32
  out = softmax((rope(xWq^T) rope(xWk^T)^T)/sqrt(128)) (xWv^T) Wo^T

Sharding (8 cores): tensor-parallel over heads x data-parallel over batch.
Core c handles batch b=c//4 and head group g=c%4 (4 of 16 heads = 512 of 2048
channels). Each core produces a full [2048, 2048] partial of the output
projection restricted to its 512 input channels; the host sums 4 partials per
batch (o_proj row-parallel reduction on host).

v4 design (v2 baseline 381.7us -> ~368-371us fast-clock, 448.9 -> ~436 on a
thermally-throttled device; the chip DVFS state is bimodal and outside our
control):
  * Everything flows in bf16 except PSUM accumulation; output partials are
    bf16 (host sums in fp32; rel_err 1.39e-2 vs 2e-2 gate). fp8 DoubleRow
    was measured on HW: per-instruction cost is IDENTICAL to bf16 (216ns
    per 512-col matmul), so DR only pays when it halves instruction count;
    pure fp8 fails accuracy (scores/value path both need >=bf16), and the
    hi/lo split costs 1.5x instructions = slower. bf16 is the floor.
  * q-projection of s-block 3 is DEFERRED into phase 2: qh[:, s3] is first
    consumed by attention column 3, so its 64 matmuls run as fillers in
    column 0's exp-paced PE bubbles (weight pool outlives phase 1; x-s3 is
    re-fetched from HBM on the idle sync ring).
  * Filler queue rationing: column 0 gets the q-s3 quads (5/block), columns
    1-3 get the previous column's o_proj groups (4/block, exactly covering
    the 12 blocks) so the exp-bound trailing blocks keep PE filler.
  * PE warmup matmuls on memset tiles at t=0 spin the DVFS p-state to full
    clock during the ~8us DMA startup (first real matmuls then run at 216ns
    instead of 400-700ns ramp).
  * Softmax denominator: wide [128,2,512] bf16 adds on Vector (9 DVE ops
    per block), one ones-matmul per block contracts the partitions; den +
    reciprocal for the final block are emitted early so only the normalize
    remains on the tail critical path.
  * Phase 2 engine discipline: ScalarE runs ONLY the exp chain; o_proj
    copies ride Vector, out DMAs ride the sync queue (gpsimd DGE is never
    used near kernel end -- its drain costs ~9us).

Per-core PE work: 768 (proj) + 256 (scores) + 256 (attn@V) + 16 (den) +
256 (o_proj) = 1552 matmuls of 512 cols at ~216ns = 328us floor at 2.4GHz;
exec ~369us = floor + ~8us head preamble + ~11us tail quiesce + transient
DMA stalls + ~5% SBUF-contention inflation in phase 2.
"""

import contextlib

import numpy as np

import concourse.mybir as mybir
import concourse.tile as tile
from concourse import bacc
from concourse.bass_utils import run_bass_kernel_spmd

F32 = mybir.dt.float32
BF16 = mybir.dt.bfloat16
F8 = mybir.dt.float8e4
AF = mybir.ActivationFunctionType

B = 2
S = 2048
HIDDEN = 2048
HEADS = 16
DH = 128
THETA = 10000.0
N_CORES = 8
GROUPS = 4
HPC = HEADS // GROUPS  # heads per core
JPC = HPC * DH  # projection cols per core
SCALE = 1.0 / np.sqrt(DH)

SB = 512
NSB = S // SB
KT = HIDDEN // 128  # 16 contraction tiles
NKT = S // 128  # 16 s_k tiles


def build():
    nc = bacc.Bacc("TRN2", target_bir_lowering=False, debug=False)

    # partition-major inputs (see _make_in_maps)
    x_d = nc.dram_tensor("xPM", [NSB, 128, KT, SB], BF16, kind="ExternalInput")
    wq_d = nc.dram_tensor("wqPM", [128, KT, JPC], BF16, kind="ExternalInput")
    wk_d = nc.dram_tensor("wkPM", [128, KT, JPC], BF16, kind="ExternalInput")
    wv_d = nc.dram_tensor("wvPM", [128, KT, JPC], BF16, kind="ExternalInput")
    wo_d = nc.dram_tensor("woPM", [128, HPC, HIDDEN], BF16, kind="ExternalInput")
    cos_d = nc.dram_tensor("cos", [DH, S], BF16, kind="ExternalInput")
    sin_d = nc.dram_tensor("sinS", [DH, S], BF16, kind="ExternalInput")
    out_d = nc.dram_tensor("out", [S, HIDDEN], BF16, kind="ExternalOutput")

    out = out_d.ap()

    with tile.TileContext(nc) as tc:
        with tc.tile_pool(name="persist", bufs=1) as persist:
            ones_sb = persist.tile([128, 128], BF16, tag="ones")
            nc.gpsimd.memset(ones_sb[:], 1.0)

            # SBUF-resident per-head q/k (transposed [dh, s]) and natural v
            qh = [
                persist.tile([128, S], BF16, tag=f"qh{h}", name=f"qh{h}")
                for h in range(HPC)
            ]
            kh = [
                persist.tile([128, S], BF16, tag=f"kh{h}", name=f"kh{h}")
                for h in range(HPC)
            ]
            vnat = [
                persist.tile([128, JPC], BF16, tag=f"v{t}", name=f"v{t}")
                for t in range(NKT)
            ]
            cos_sb = persist.tile([128, S], BF16, tag="cos")
            sin_sb = persist.tile([128, S], BF16, tag="sin")
            wo = persist.tile([128, HPC, HIDDEN], BF16, tag="wo")
            yt = persist.tile([128, HPC, S], BF16, tag="yt")

            # ---------------- Phase 1: projections + RoPE ------------------
            # s-block outer, projection inner: x is streamed from HBM once.
            # Chunked loads (finest first) so the first matmuls start as soon
            # as the DMA rings deliver data (~10us fixed startup latency).
            NCH = 5
            CHS = (1, 1, 2, 4, 8)  # weight chunk sizes
            CHO = (0, 1, 2, 4, 8)  # weight chunk offsets
            NCHX = 6
            CHSX = (1, 1, 2, 4, 4, 4)  # x chunk sizes (k8-11 ride sync ring)
            CHOX = (0, 1, 2, 4, 8, 12)

            def wslice(chunks, k, cols, chs=CHS, cho=CHO):
                for c in range(len(chs)):
                    if k < cho[c] + chs[c]:
                        return chunks[c][:, k - cho[c], cols]
                raise AssertionError

            # p1w (weight chunks) outlives phase 1: the deferred q-s3
            # projection consumes wq from inside phase 2 (see below)
            p1w_stack = contextlib.ExitStack()
            p1w = p1w_stack.enter_context(tc.tile_pool(name="p1w", bufs=1))
            xq3 = persist.tile([128, KT, SB], BF16, tag="xq3")
            with (
                tc.tile_pool(name="p1x", bufs=2) as p1x,
                tc.tile_pool(name="p1s", bufs=6) as p1s,
                tc.tile_pool(name="p1ps", bufs=4, space="PSUM") as p1ps,
                tc.tile_pool(name="p1wps", bufs=1, space="PSUM") as p1wps,
            ):
                # PE warmup: matmuls on memset tiles spin the Tensor engine to
                # its full p-state during the ~10us DMA startup latency, so the
                # first real matmuls run at 216ns instead of 400-700ns (DVFS
                # ramp). Extra bursts fill the early DMA-bound stalls.
                wdum = p1s.tile([128, 128], BF16, tag="wdum")
                xdum = p1s.tile([128, SB], BF16, tag="xdum")
                wps = p1wps.tile([128, SB], F32, tag="wps")
                nc.vector.memset(wdum[:], 0.0)
                nc.vector.memset(xdum[:], 0.0)

                def warm(n):
                    for _ in range(n):
                        nc.tensor.matmul(
                            wps[:], wdum[:], xdum[:], start=True, stop=True
                        )

                wchunks = {"q": [], "k": [], "v": []}

                def load_w(name, w_d, c, eng):
                    w = p1w.tile([128, CHS[c], JPC], BF16, tag=f"w{name}{c}")
                    eng.dma_start(
                        out=w[:], in_=w_d.ap()[:, CHO[c] : CHO[c] + CHS[c], :]
                    )
                    wchunks[name].append(w)

                def load_xs(s, engs=None):
                    xsc = []
                    for c in range(NCHX):
                        eng = nc.scalar if engs is None else engs[c]
                        xt = p1x.tile([128, CHSX[c], SB], BF16, tag=f"xs{c}")
                        eng.dma_start(
                            out=xt[:],
                            in_=x_d.ap()[s, :, CHOX[c] : CHOX[c] + CHSX[c], :],
                        )
                        xsc.append(xt)
                    return xsc

                # startup ordering: j0's operands (wq + x0) split across both
                # DMA rings so they arrive at aggregate HBM bandwidth; x0's
                # k8-11 chunk rides the sync ring behind wq to balance bytes.
                # (Further shuffles tested flat: the early phase is pinned by
                # HBM bandwidth + DGE outstanding-descriptor windows.)
                warm(40)
                for c in range(NCH):
                    load_w("q", wq_d, c, nc.sync)
                # x0 entirely on the scalar ring: the sync ring carries only
                # wq+wk so the k groups at ~20us never starve
                xs_next = load_xs(0, (nc.scalar,) * NCHX)
                # cos/sin for s-block 0 only (tiny), rest after wv
                sb0 = slice(0, SB)
                nc.scalar.dma_start(out=cos_sb[:, sb0], in_=cos_d.ap()[:, sb0])
                nc.scalar.dma_start(out=sin_sb[:, sb0], in_=sin_d.ap()[:, sb0])
                # wk split across BOTH rings: c0-c2 (k0-3) land on scalar
                # right after x0, c3-c4 on sync behind wq, so the k groups at
                # ~17us start with no stall
                for c in range(3):
                    load_w("k", wk_d, c, nc.scalar)
                for c in range(3, NCH):
                    load_w("k", wk_d, c, nc.sync)
                # wv likewise split: c0-c2 behind wk on scalar, c3-c4 behind
                # wk on sync -- its tail chunk otherwise lands right at the
                # v-groups' deadline (~24us) and stalls them ~1.5-2.5us
                for c in range(3):
                    load_w("v", wv_d, c, nc.scalar)
                for c in range(3, NCH):
                    load_w("v", wv_d, c, nc.sync)
                rest = slice(SB, S)
                nc.scalar.dma_start(out=cos_sb[:, rest], in_=cos_d.ap()[:, rest])
                nc.scalar.dma_start(out=sin_sb[:, rest], in_=sin_d.ap()[:, rest])

                # s=3: q is DEFERRED to phase 2 (qh[:, s3] is not consumed
                # until attention column 3, ~150us into phase 2) where its 64
                # matmuls fill column 0's exp-paced PE bubbles. k goes first
                # so kh[h] is complete for phase 2's first attention block.
                order_qkv = [("q", j) for j in range(HPC)]
                order_qkv += [("k", j) for j in range(HPC)]
                order_qkv += [("v", j) for j in range(HPC)]
                order_int = [("k", j) for j in range(HPC)]
                order_int += [("v", j) for j in range(HPC)]

                for s in range(NSB):
                    sblk = slice(s * SB, (s + 1) * SB)
                    xsc = xs_next
                    if s + 1 < NSB and s > 0:
                        # prefetch on the sync ring: FIFO order naturally
                        # deprioritizes it behind the critical weight loads
                        xs_next = load_xs(s + 1, (nc.sync,) * NCHX)
                    for gi, (name, j) in enumerate(
                        order_int if s == NSB - 1 else order_qkv
                    ):
                        if s == 0 and gi == 8:
                            # x1 prefetch deferred past the weight loads: it
                            # is not needed until ~52us, and 2MB ahead of wk
                            # on the sync ring starved the k groups at ~23us
                            xs_next = load_xs(1, (nc.sync,) * NCHX)
                        dst = {"q": qh, "k": kh, "v": None}[name]
                        jblk = slice(j * 128, (j + 1) * 128)
                        ps = p1ps.tile([128, SB], F32, tag="ps")
                        for k in range(KT):
                            if dst is not None:  # Q/K: [j, s] transposed
                                lhsT = wslice(wchunks[name], k, jblk)
                                rhs = wslice(xsc, k, slice(0, SB), CHSX, CHOX)
                            else:  # V: natural [s, j]
                                lhsT = wslice(xsc, k, jblk, CHSX, CHOX)
                                rhs = wslice(wchunks[name], k, slice(0, JPC))
                            nc.tensor.matmul(
                                ps[:],
                                lhsT,
                                rhs,
                                start=(k == 0),
                                stop=(k == KT - 1),
                            )
                        if dst is not None:
                            qt = p1s.tile([128, SB], BF16, tag="qt")
                            tmp = p1s.tile([128, SB], BF16, tag="tmp")
                            nc.scalar.copy(qt[:], ps[:])
                            # rotate-half swaps are SBUF->SBUF: issue them on
                            # the idle gpsimd DMA queue so x-prefetch issue
                            # slices on the sync ring cannot delay the RoPE
                            # chain (which backpressures PE via tile pools)
                            nc.gpsimd.dma_start(
                                out=tmp[0:64, :], in_=qt[64:128, :]
                            )
                            nc.gpsimd.dma_start(
                                out=tmp[64:128, :], in_=qt[0:64, :]
                            )
                            nc.vector.tensor_mul(qt[:], qt[:], cos_sb[:, sblk])
                            nc.vector.tensor_mul(tmp[:], tmp[:], sin_sb[:, sblk])
                            nc.vector.tensor_add(dst[j][:, sblk], qt[:], tmp[:])
                        else:
                            nc.scalar.copy(vnat[s * HPC + j][:], ps[:])

            # -------- Phase 2+3: attention with fused o_proj ---------------
            # n-outer block order: after column n's 4 heads, yt[:, :, nblk]
            # is complete, so column n's o_proj groups (4 matmuls each) are
            # interleaved into column n+1's attention blocks — they fill the
            # PE idle slots while ScalarE runs exp. PSUM: scores 4 + num 2 +
            # den 1 + o_proj 1 = 8 banks. The last column's o_proj runs after
            # phase 2 with all banks free.
            NOUT = HIDDEN // SB
            PIPE = 3
            NP = NKT // 2  # 8 score pairs

            def p3_group(m, nn, psp, ocp, ci):
                # copies on vector/gpsimd and DMAs on sync: ScalarE stays
                # dedicated to the exp chain in phase 2
                mblk = slice(m * 128, (m + 1) * 128)
                nblk = slice(nn * SB, (nn + 1) * SB)
                ps = psp.tile([128, SB], F32, tag="p3ps", name="p3ps")
                for kj in range(HPC):
                    nc.tensor.matmul(
                        ps[:],
                        yt[:, kj, mblk],
                        wo[:, kj, nblk],
                        start=(kj == 0),
                        stop=(kj == HPC - 1),
                    )
                oc = ocp.tile([128, SB], BF16, tag="oc", name="oc")
                # copies on vector only: a copy on ScalarE delays the in-order
                # exp chain (measured +5us)
                nc.vector.tensor_copy(oc[:], ps[:])
                # all out DMAs ride the sync queue (idle in phase 2); gpsimd
                # DMA is avoided near kernel end: its dge_drain costs ~9us
                nc.sync.dma_start(out=out[mblk, nblk], in_=oc[:])

            with (
                tc.tile_pool(name="p2e", bufs=6) as p2e,
                tc.tile_pool(name="p2ac", bufs=3) as p2ac,
                tc.tile_pool(name="p2t", bufs=3) as p2t,
                tc.tile_pool(name="p2r", bufs=3) as p2r,
                tc.tile_pool(name="p3s", bufs=6) as p3s,
                tc.tile_pool(name="p2sc", bufs=2, space="PSUM") as p2sc,
                tc.tile_pool(name="p2num", bufs=2, space="PSUM") as p2num,
                tc.tile_pool(name="p2den", bufs=1, space="PSUM") as p2den,
                tc.tile_pool(name="p3ps", bufs=1, space="PSUM") as p3ps,
            ):
                pending = None  # (acc, num, den, h, nblk, r_pre) of prev block

                def flush_pending():
                    acc, num, den, ph, pnblk, r_pre = pending
                    if r_pre is None:
                        nc.tensor.matmul(
                            den[:], ones_sb[:], acc[:], start=True, stop=True
                        )
                        r = p2r.tile([128, SB], F32, tag="r")
                        scr = p2r.tile([128, SB], F32, tag="scr")
                        nc.vector.reciprocal_approx_accurate(
                            out=r[:], in_=den[:], scratch=scr[:]
                        )
                    else:
                        r = r_pre
                    nc.vector.tensor_mul(yt[:, ph, pnblk], num[:], r[:])

                # Filler queue: thunks of ~4 matmuls each that fill the PE
                # slots left idle by exp pacing. Column 0 gets the deferred
                # q-s3 projection quads; columns 1+ get the previous column's
                # o_proj groups. Strict FIFO order keeps the shared p3ps bank
                # (bufs=1) conflict-free.
                fillers = []
                s3blk = slice(3 * SB, 4 * SB)
                q3ps = {}

                def q3_quad(j, qi):
                    jblk = slice(j * 128, (j + 1) * 128)

                    def thunk():
                        if qi == 0:
                            q3ps[j] = p3ps.tile(
                                [128, SB], F32, tag="p3ps", name="p3ps"
                            )
                        ps = q3ps[j]
                        for k in range(4 * qi, 4 * qi + 4):
                            nc.tensor.matmul(
                                ps[:],
                                wslice(wchunks["q"], k, jblk),
                                xq3[:, k, :],
                                start=(k == 0),
                                stop=(k == KT - 1),
                            )
                        if qi == 3:
                            # RoPE finish (phase-1 recipe; copies on vector,
                            # ScalarE stays on exp)
                            qt = p3s.tile([128, SB], BF16, tag="qt3", name="qt3")
                            tmp = p3s.tile(
                                [128, SB], BF16, tag="tmp3", name="tmp3"
                            )
                            nc.vector.tensor_copy(qt[:], ps[:])
                            nc.gpsimd.dma_start(
                                out=tmp[0:64, :], in_=qt[64:128, :]
                            )
                            nc.gpsimd.dma_start(
                                out=tmp[64:128, :], in_=qt[0:64, :]
                            )
                            nc.vector.tensor_mul(qt[:], qt[:], cos_sb[:, s3blk])
                            nc.vector.tensor_mul(
                                tmp[:], tmp[:], sin_sb[:, s3blk]
                            )
                            nc.vector.tensor_add(
                                qh[j][:, s3blk], qt[:], tmp[:]
                            )

                    return thunk

                for j3 in range(HPC):
                    for qi in range(4):
                        fillers.append(q3_quad(j3, qi))

                p3ci = [0]

                def make_p3(m, nn):
                    def thunk():
                        p3_group(m, nn, p3ps, p3s, p3ci[0])
                        p3ci[0] += 1

                    return thunk

                for n in range(NSB):
                    nblk = slice(n * SB, (n + 1) * SB)
                    if n >= 1:
                        for m in range(HPC * (n - 1), HPC * n):
                            for nn in range(NOUT):
                                fillers.append(make_p3(m, nn))
                    for h in range(HPC):
                        hblk = slice(h * 128, (h + 1) * 128)
                        if n == 0 and h == 0:
                            # xq3 first (q-s3 fillers start at h=1), then wo
                            # (first needed at column 1, ~44us in)
                            nc.sync.dma_start(out=xq3[:], in_=x_d.ap()[3])
                            for kj in range(HPC):
                                nc.sync.dma_start(
                                    out=wo[:, kj, :], in_=wo_d.ap()[:, kj, :]
                                )
                        # n=0 (h>=1): drain the 16 q-s3 quads at 5/block; then
                        # 4/block spreads the 48 o_proj groups exactly over
                        # the remaining 12 blocks
                        bi = n * HPC + h
                        quota = 0 if bi == 0 else (5 if n == 0 else 4)
                        num = p2num.tile([128, SB], F32, tag="num")
                        den = p2den.tile([128, SB], F32, tag="den")
                        acc = p2ac.tile([128, 2, SB], BF16, tag="acc")
                        accf = None
                        last = n == NSB - 1 and h == HPC - 1
                        r_last = None
                        es = [None] * NP
                        for p in range(NP + PIPE):
                            if p < NP:
                                sc2 = p2sc.tile([128, 2, SB], F32, tag="sc")
                                e2 = p2e.tile([128, 2, SB], BF16, tag="e")
                                for half in range(2):
                                    i = 2 * p + half
                                    nc.tensor.matmul(
                                        sc2[:, half, :],
                                        kh[h][:, i * 128 : (i + 1) * 128],
                                        qh[h][:, nblk],
                                        start=True,
                                        stop=True,
                                    )
                                nc.scalar.activation(
                                    e2[:], sc2[:], AF.Exp, scale=float(SCALE)
                                )
                                es[p] = e2
                                # wide esum: one [128,2,512] add per pair (9
                                # DVE ops/block vs 15), folded once at p==NP
                                if p == 0:
                                    nc.vector.tensor_copy(acc[:], e2[:])
                                else:
                                    nc.vector.tensor_add(
                                        acc[:], acc[:], e2[:]
                                    )
                            if p == PIPE + 1 and pending is not None:
                                # previous block's den matmul + normalize,
                                # emitted late so PE never waits on the DVE
                                # esum chain
                                flush_pending()
                            if p == NP:
                                accf = p2t.tile([128, SB], BF16, tag="t")
                                nc.vector.tensor_add(
                                    accf[:], acc[:, 0, :], acc[:, 1, :]
                                )
                            if last and p == NP:
                                # final block: den + recip as soon as the esum
                                # is complete, so only the normalize remains on
                                # the tail critical path after the last num mm
                                nc.tensor.matmul(
                                    den[:], ones_sb[:], accf[:],
                                    start=True, stop=True,
                                )
                                r_last = p2r.tile([128, SB], F32, tag="r")
                                scr = p2r.tile([128, SB], F32, tag="scr")
                                nc.vector.reciprocal_approx_accurate(
                                    out=r_last[:], in_=den[:], scratch=scr[:]
                                )
                            if p >= PIPE:
                                pp = p - PIPE
                                for half in range(2):
                                    i = 2 * pp + half
                                    nc.tensor.matmul(
                                        num[:],
                                        vnat[i][:, hblk],
                                        es[pp][:, half, :],
                                        start=(i == 0),
                                        stop=(i == NKT - 1),
                                    )
                            if (
                                PIPE + 2 <= p < PIPE + 2 + quota
                                and fillers
                            ):
                                # one ~4-matmul filler per slot, rationed so
                                # the queue lasts through the final exp-bound
                                # blocks (greedy FIFO starves the tail)
                                fillers.pop(0)()
                        pending = (accf, num, den, h, nblk, r_last)
                # final normalize FIRST: it gates the whole post-loop, and a
                # leftover filler's oc-copy ahead of it in the vector queue
                # costs the PE ~1.5us
                flush_pending()
                while fillers:
                    fillers.pop(0)()

            # weight chunks are dead once the q-s3 fillers ran; release the
            # pool before the post-loop so its drain cost lands mid-kernel
            p1w_stack.close()

            # last column's o_proj: phase-2 PSUM pools are closed, use dense
            # kj-outer groups across 4 banks with overlapped evacuation
            with (
                tc.tile_pool(name="p3sb", bufs=2) as p3sb,
                tc.tile_pool(name="p3psb", bufs=2, space="PSUM") as p3psb,
            ):
                # copies round-robin vector/gpsimd/scalar, DMAs round-robin
                # all four queues: the tail drains as fast as possible
                cengs = (nc.vector, nc.scalar)
                dengs = (nc.sync, nc.scalar)
                ci = 0
                for m in range(S // 128 - HPC, S // 128):
                    mblk = slice(m * 128, (m + 1) * 128)
                    ps4 = [
                        p3psb.tile([128, SB], F32, tag=f"ps{nn}", name=f"ps{nn}")
                        for nn in range(NOUT)
                    ]
                    # nn-outer so each PSUM tile finishes early and its
                    # copy/DMA overlaps remaining matmuls
                    for nn in range(NOUT):
                        nblk = slice(nn * SB, (nn + 1) * SB)
                        for kj in range(HPC):
                            nc.tensor.matmul(
                                ps4[nn][:],
                                yt[:, kj, mblk],
                                wo[:, kj, nblk],
                                start=(kj == 0),
                                stop=(kj == HPC - 1),
                            )
                        oc = p3sb.tile(
                            [128, SB], BF16, tag=f"oc{nn}", name=f"oc{nn}"
                        )
                        ceng = cengs[ci % len(cengs)]
                        if ceng is nc.scalar:
                            ceng.copy(oc[:], ps4[nn][:])
                        else:
                            ceng.tensor_copy(oc[:], ps4[nn][:])
                        dengs[ci % len(dengs)].dma_start(
                            out=out[mblk, nblk], in_=oc[:]
                        )
                        ci += 1

    nc.compile()
    return nc


_NC = None


def _get_nc():
    global _NC
    if _NC is None:
        _NC = build()
    return _NC


BF16_NP = np.dtype(mybir.dt.np(BF16))


def _rope_tables():
    inv_freq = 1.0 / (THETA ** (np.arange(0, DH, 2, dtype=np.float32) / DH))
    freqs = np.arange(S, dtype=np.float32)[:, None] * inv_freq[None, :]  # [S, 64]
    cos_h = np.cos(freqs).T.astype(np.float32)  # [64, S]
    sin_h = np.sin(freqs).T.astype(np.float32)
    cos = np.concatenate([cos_h, cos_h], axis=0)  # [128, S]
    sin_s = np.concatenate([-sin_h, sin_h], axis=0)  # [128, S]
    return (
        np.ascontiguousarray(cos).astype(BF16_NP),
        np.ascontiguousarray(sin_s).astype(BF16_NP),
    )


def _pm_weight(wT):  # [2048, 512] (k, j) -> [128, 16, 512] partition-major
    return np.ascontiguousarray(
        wT.reshape(KT, 128, JPC).transpose(1, 0, 2)
    ).astype(BF16_NP)


def _make_in_maps(inputs):
    x = np.asarray(inputs["x"], dtype=np.float32)
    Wq = np.asarray(inputs["Wq"], dtype=np.float32)
    Wk = np.asarray(inputs["Wk"], dtype=np.float32)
    Wv = np.asarray(inputs["Wv"], dtype=np.float32)
    Wo = np.asarray(inputs["Wo"], dtype=np.float32)

    cos, sin_s = _rope_tables()

    in_maps = []
    for c in range(N_CORES):
        b = c // GROUPS
        g = c % GROUPS
        rows = slice(g * JPC, (g + 1) * JPC)
        xT = x[b].T  # [hidden(k), s]
        # [k, s] -> [s_blk, p, kt, s_in_blk]
        xpm = np.ascontiguousarray(
            xT.reshape(KT, 128, NSB, SB).transpose(2, 1, 0, 3)
        ).astype(BF16_NP)
        # Wo[:, rows].T -> [512(j), 2048] -> [p, kj, 2048]
        woT = Wo[:, rows].T
        wopm = np.ascontiguousarray(
            woT.reshape(HPC, 128, HIDDEN).transpose(1, 0, 2)
        ).astype(BF16_NP)
        in_maps.append(
            {
                "xPM": xpm,
                "wqPM": _pm_weight(Wq[rows].T),
                "wkPM": _pm_weight(Wk[rows].T),
                "wvPM": _pm_weight(Wv[rows].T),
                "woPM": wopm,
                "cos": cos,
                "sinS": sin_s,
            }
        )
    return in_maps


def kernel(x, Wq, Wk, Wv, Wo):
    nc = _get_nc()
    in_maps = _make_in_maps({"x": x, "Wq": Wq, "Wk": Wk, "Wv": Wv, "Wo": Wo})
    res = run_bass_kernel_spmd(nc, in_maps, list(range(N_CORES)))

    out = np.zeros((B, S, HIDDEN), dtype=np.float32)
    for c in range(N_CORES):
        out[c // GROUPS] += res.results[c]["out"].astype(np.float32)
    return out

